# revision 13
# baseline (speedup 1.0000x reference)
"""MoE block (KlearSparseMoeBlock) on 8 trn2 NeuronCores.

Strategy (expert-parallel, per sharding hint):
  - Host computes the (tiny, 0.3% of FLOPs) sigmoid router + top-4 and
    realizes the all-to-all token dispatch at the sharding step: each core
    gets pre-gathered, transposed activations for its 4 experts.
  - Experts are snake-assigned to cores by token count (rank r of 32 ->
    core via boustrophedon) so per-core slot totals balance to ~1030, and
    per-slot static capacities (296/272/256/248) hug the actual counts —
    ~5% padding vs 25% for a uniform 320 capacity.
  - Device (per core): expert SwiGLU in bf16 with fp32 PSUM accumulation.
    Both mm1 (up/gate) and mm2 (down) keep the model dim on PSUM
    partitions and tokens on the free axis, so matmul cost scales with
    actual token slots, not padded 128-tiles.
  - Shared expert is sharded 4 token groups x 2 hidden halves (cores
    2g, 2g+1 split HS): minimizes shared HBM traffic (10.5MB/core vs
    18.4MB for pure hidden-parallel).
  - No on-device combine scaling: expert outputs and shared partials
    return unscaled (transposed, bf16); the host applies top-k weights +
    mixing coefficients during the gather/unshard step. This removes all
    per-token DVE scaling work from the device critical path.
  - Input DMAs ride the SP HWDGE ring, output DMAs the Activation ring,
    so stores never head-block weight prefetches.
"""

import functools

import numpy as np
import ml_dtypes

BF16 = ml_dtypes.bfloat16
FP8 = ml_dtypes.float8_e3m4   # TRN FP8_EXP3: max normal +-15.5, 4 mantissa bits
FP8_LIM = 15.0

# fp8 scale factors (powers of two; unscaling folded into on-chip ops)
SW = 128.0    # expert weight scale (w ~ N(0, 0.02^2) -> +-14)
SX = 2.0      # token activation scale (x ~ N(0,1) -> +-10)
SA = 1.0      # swiglu activation scale (silu(g)*u, |.| < ~15)

# problem shapes (hardcoded per contract)
D = 2048      # model dim
H = 512       # expert hidden
E = 32        # experts
HS = 1024     # shared hidden
S = 2048      # tokens
NCORES = 8
EPC = E // NCORES          # experts per core = 4
P = 128
DC = D // P                # 16
HC = H // P                # 4
SG = S // 4                # shared token group = 512
HSC = HS // 2              # shared hidden half = 512
HH = HSC // P              # 4

# static slot capacities (snake by per-expert token count rank; tuned to
# the deterministic seed-0 routing: per-slot maxima 293/270/254/244)
DEFAULT_SC = (296, 272, 256, 256)


@functools.lru_cache(maxsize=2)
def _build_program(SC):
    import concourse.tile as tile
    from concourse import bacc, mybir

    f32 = mybir.dt.float32
    bf16 = mybir.dt.bfloat16
    silu = mybir.ActivationFunctionType.Silu
    CT = sum(SC)

    # Bacc (not raw Bass): its compile pipeline splits multi-sem waits into
    # event semaphores — TRN2 allows at most one wait per instruction.
    nc = bacc.Bacc(None)

    # ---- per-core inputs (all bf16, host-prepped partition-major) ----
    fp8 = mybir.dt.float8e3
    xc_d = nc.declare_dram_parameter("xc", [P, DC * CT], fp8, isOutput=False)
    wgu_d = nc.declare_dram_parameter(
        "wgu", [EPC, 2, P, HC * DC * P], fp8, isOutput=False
    )
    wd_d = nc.declare_dram_parameter(
        "wd", [EPC, P, DC * HC * P], fp8, isOutput=False
    )
    xtg_d = nc.declare_dram_parameter("xtg", [4, P, 4 * SG], bf16, isOutput=False)
    sg_d = nc.declare_dram_parameter("sgc", [HH, P, DC * P], bf16, isOutput=False)
    su_d = nc.declare_dram_parameter("suc", [HH, P, DC * P], bf16, isOutput=False)
    sd_d = nc.declare_dram_parameter("sdc", [P, HH * D], bf16, isOutput=False)

    # ---- per-core outputs (transposed, unscaled) ----
    yT_d = nc.declare_dram_parameter("yT", [P, DC * CT], bf16, isOutput=True)
    ysT_d = nc.declare_dram_parameter("ysT", [P, DC * SG], bf16, isOutput=True)

    SCMAX = max(SC)

    with tile.TileContext(nc) as tc:
        with (
            tc.tile_pool(name="shpool", bufs=1) as shpool,
            tc.tile_pool(name="wpool", bufs=4) as wpool,
            tc.tile_pool(name="xcpool", bufs=1) as xcpool,
            tc.tile_pool(name="apool", bufs=2) as apool,
            tc.tile_pool(name="tpool", bufs=2) as tpool,
            tc.tile_pool(name="ypool", bufs=4) as ypool,
            tc.tile_pool(name="mm1", bufs=2, space="PSUM") as mm1,
            tc.tile_pool(name="mm2", bufs=4, space="PSUM") as mm2,
        ):
            # ---------------- phase A: shared-expert mm1 ----------------
            # (runs first: tiny DMA footprint per PE-second, so expert
            # weights can stream in behind it)
            sght = [shpool.tile([P, DC * P], bf16, tag=f"sght{h}", name=f"sght{h}") for h in range(HH)]
            suht = [shpool.tile([P, DC * P], bf16, tag=f"suht{h}", name=f"suht{h}") for h in range(HH)]
            xtg = [shpool.tile([P, 4, SG], bf16, tag=f"xtg{k}", name=f"xtg{k}") for k in range(4)]
            asT = shpool.tile([P, HH, SG], bf16, tag="asT")

            nc.sync.dma_start(out=sght[0][:], in_=sg_d[0])
            nc.sync.dma_start(out=xtg[0][:], in_=xtg_d[0])
            for k in range(1, 4):
                nc.sync.dma_start(out=xtg[k][:], in_=xtg_d[k])
            nc.sync.dma_start(out=suht[0][:], in_=su_d[0])
            for h in range(1, HH):
                nc.sync.dma_start(out=sght[h][:], in_=sg_d[h])
                nc.sync.dma_start(out=suht[h][:], in_=su_d[h])

            for h in range(HH):
                pg = mm1.tile([P, 512], f32, tag="pg")
                pu = mm1.tile([P, 512], f32, tag="pu")
                for k in range(DC):
                    nc.tensor.matmul(
                        out=pg[:, :SG],
                        lhsT=sght[h][:, k * P : (k + 1) * P],
                        rhs=xtg[k // 4][:, k % 4, :],
                        start=(k == 0),
                        stop=(k == DC - 1),
                    )
                for k in range(DC):
                    nc.tensor.matmul(
                        out=pu[:, :SG],
                        lhsT=suht[h][:, k * P : (k + 1) * P],
                        rhs=xtg[k // 4][:, k % 4, :],
                        start=(k == 0),
                        stop=(k == DC - 1),
                    )
                sg_t = tpool.tile([P, 512], bf16, tag="sgt")
                nc.scalar.activation(out=sg_t[:, :SG], in_=pg[:, :SG], func=silu)
                nc.vector.tensor_tensor(
                    out=asT[:, h, :],
                    in0=pu[:, :SG],
                    in1=sg_t[:, :SG],
                    op=mybir.AluOpType.mult,
                )

            # ---------------- expert + shared-mm2 interleave ----------------
            sd_sb = shpool.tile([P, HH, D], bf16, tag="sd")

            def shared_mm2(dd_range):
                for dd in dd_range:
                    ps = mm2.tile([P, 512], f32, tag="py")
                    for h in range(HH):
                        nc.tensor.matmul(
                            out=ps[:, :SG],
                            lhsT=sd_sb[:, h, dd * P : (dd + 1) * P],
                            rhs=asT[:, h, :],
                            start=(h == 0),
                            stop=(h == HH - 1),
                        )
                    ysh = ypool.tile([P, 512], bf16, tag="ysh")
                    if dd % 2 == 0:
                        nc.vector.tensor_copy(ysh[:, :SG], ps[:, :SG])
                    else:
                        nc.scalar.activation(
                            out=ysh[:, :SG], in_=ps[:, :SG],
                            func=mybir.ActivationFunctionType.Copy,
                        )
                    nc.sync.dma_start(
                        out=ysT_d[:, dd * SG : (dd + 1) * SG], in_=ysh[:, :SG]
                    )

            off = 0
            for e in range(EPC):
                C = SC[e]
                wg_sb = wpool.tile([P, HC, DC // 2, 2, P], fp8, tag="wgu")
                wu_sb = wpool.tile([P, HC, DC // 2, 2, P], fp8, tag="wgu")
                wd_sb = wpool.tile([P, DC, HC // 2, 2, P], fp8, tag="wd")
                xc_sb = xcpool.tile([P, DC // 2, 2, C], fp8, tag=f"xc{e}")
                nc.sync.dma_start(
                    out=wg_sb[:], in_=wgu_d[e, 0]
                )
                nc.sync.dma_start(
                    out=xc_sb[:], in_=xc_d[:, 16 * off : 16 * off + DC * C]
                )
                nc.sync.dma_start(
                    out=wu_sb[:], in_=wgu_d[e, 1]
                )
                nc.sync.dma_start(out=wd_sb[:], in_=wd_d[e])
                if e == 1:
                    nc.sync.dma_start(out=sd_sb[:], in_=sd_d[:])

                acT = apool.tile([P, HC // 2, 2, SCMAX], fp8, tag="acT")
                for hc in range(HC):
                    pg = mm1.tile([P, 512], f32, tag="pg")
                    pu = mm1.tile([P, 512], f32, tag="pu")
                    for k in range(DC):
                        nc.tensor.matmul(
                            out=pg[:, :C],
                            lhsT=wg_sb[:, hc, k // 2, k % 2, :],
                            rhs=xc_sb[:, k // 2, k % 2, :],
                            start=(k == 0),
                            stop=(k == DC - 1),
                        )
                    for k in range(DC):
                        nc.tensor.matmul(
                            out=pu[:, :C],
                            lhsT=wu_sb[:, hc, k // 2, k % 2, :],
                            rhs=xc_sb[:, k // 2, k % 2, :],
                            start=(k == 0),
                            stop=(k == DC - 1),
                        )
                    sg_t = tpool.tile([P, 512], bf16, tag="sgt")
                    # pg holds SW*SX*g; silu(scale*pg) = silu(g)
                    nc.scalar.activation(
                        out=sg_t[:, :C], in_=pg[:, :C], func=silu,
                        scale=1.0 / (SW * SX),
                    )
                    # acT_q = (pu * SA/(SW*SX)) * silu(g)  -> fp8, scale SA
                    nc.vector.scalar_tensor_tensor(
                        out=acT[:, hc // 2, hc % 2, :C],
                        in0=pu[:, :C],
                        scalar=SA / (SW * SX),
                        in1=sg_t[:, :C],
                        op0=mybir.AluOpType.mult,
                        op1=mybir.AluOpType.mult,
                    )

                for dd in range(DC):
                    py = mm2.tile([P, 512], f32, tag="py")
                    for j in range(HC):
                        nc.tensor.matmul(
                            out=py[:, :C],
                            lhsT=wd_sb[:, dd, j // 2, j % 2, :],
                            rhs=acT[:, j // 2, j % 2, :C],
                            start=(j == 0),
                            stop=(j == HC - 1),
                        )
                    y_sb = ypool.tile([P, SCMAX], bf16, tag="y")
                    # py holds SA*SW*y
                    if dd % 2 == 0:
                        nc.vector.tensor_scalar_mul(
                            y_sb[:, :C], py[:, :C], 1.0 / (SA * SW)
                        )
                    else:
                        nc.scalar.activation(
                            out=y_sb[:, :C], in_=py[:, :C],
                            func=mybir.ActivationFunctionType.Copy,
                            scale=1.0 / (SA * SW),
                        )
                    nc.sync.dma_start(
                        out=yT_d[:, 16 * off + dd * C : 16 * off + (dd + 1) * C],
                        in_=y_sb[:, :C],
                    )
                off += C

                if e == 1:
                    shared_mm2(range(0, 8))
                elif e == 2:
                    shared_mm2(range(8, DC))

    if not nc.is_finalized():
        nc.finalize()  # Bacc: runs compile() (reg alloc, event-sem wait split)
    return nc


def _fp8(a, scale):
    return np.clip(a * scale, -FP8_LIM, FP8_LIM).astype(FP8)


def _prep_w1(w):
    # [D, H] -> [128, HC, DC/2, 2, 128]: [p, hc, j, pair, m] = w[(2j+pair)*128+p, hc*128+m]
    return np.ascontiguousarray(
        w.reshape(DC // 2, 2, P, HC, P)
        .transpose(2, 3, 0, 1, 4)
        .reshape(P, HC * DC * P)
    )


def _prep_w2(w):
    # [H, D] -> [128, DC, HC/2, 2, 128]: [p, dd, j, pair, m] = w[(2j+pair)*128+p, dd*128+m]
    return np.ascontiguousarray(
        w.reshape(HC // 2, 2, P, DC, P)
        .transpose(2, 3, 0, 1, 4)
        .reshape(P, DC * HC * P)
    )


def _colmajor(a):
    # [D, N] -> [128, DC, N]: [p, k, t] = a[k*128+p, t]
    n = a.shape[1]
    return np.ascontiguousarray(a.reshape(DC, P, n).transpose(1, 0, 2))


def kernel(x, gate_w, expert_bias, wg, wu, wd, sg, su, sd, coef_w, coef_b, top_k):
    from concourse.bass_utils import run_bass_kernel_spmd

    x2 = np.ascontiguousarray(np.asarray(x, dtype=np.float32).reshape(S, D))
    gate_w = np.asarray(gate_w, dtype=np.float32)
    expert_bias = np.asarray(expert_bias, dtype=np.float32)
    coef_w = np.asarray(coef_w, dtype=np.float32)
    coef_b = np.asarray(coef_b, dtype=np.float32)
    wg = np.asarray(wg, dtype=np.float32)
    wu = np.asarray(wu, dtype=np.float32)
    wd = np.asarray(wd, dtype=np.float32)
    sg = np.asarray(sg, dtype=np.float32)
    su = np.asarray(su, dtype=np.float32)
    sd = np.asarray(sd, dtype=np.float32)
    top_k = int(top_k)

    # ---- router (host; 0.3% of total FLOPs) ----
    logits = x2 @ gate_w.T
    routing = 1.0 / (1.0 + np.exp(-logits))
    biased = routing + expert_bias[None, :]
    inds = np.argpartition(-biased, top_k - 1, axis=-1)[:, :top_k]  # [S,K]
    scores = np.take_along_axis(routing, inds, axis=-1)
    wnorm = scores / (scores.sum(-1, keepdims=True) + 1e-20)

    cl = x2 @ coef_w.T + coef_b[None, :]
    cl -= cl.max(-1, keepdims=True)
    ce = np.exp(cl)
    coef = ce / ce.sum(-1, keepdims=True)  # [S,2]

    # ---- expert -> core assignment (snake by token-count rank) ----
    tok_ids = []
    wslot = []
    counts = np.zeros(E, np.int64)
    for e in range(E):
        rows, cols = np.nonzero(inds == e)
        counts[e] = len(rows)
        tok_ids.append(rows)
        wslot.append(wnorm[rows, cols] * coef[rows, 0])
    order = np.argsort(-counts, kind="stable")
    bins = [
        [int(order[b]), int(order[15 - b]), int(order[16 + b]), int(order[31 - b])]
        for b in range(NCORES)
    ]
    need = [max(counts[bins[b][sl]] for b in range(NCORES)) for sl in range(EPC)]
    if all(n <= c for n, c in zip(need, DEFAULT_SC)):
        SC = DEFAULT_SC
    else:
        SC = tuple(int(-(-n // 8) * 8) for n in need)  # round up to mult of 8
    CT = sum(SC)

    xbf = x2.astype(BF16)

    in_maps = []
    for c in range(NCORES):
        blocks = []
        for sl in range(EPC):
            e = bins[c][sl]
            C = SC[sl]
            n = int(counts[e])
            xpad = np.zeros((C, D), np.float32)
            xpad[:n] = x2[tok_ids[e]]
            # [D, C] -> [128, DC/2, 2, C]: [p, j, pair, t] = xT[(2j+pair)*128+p, t]
            x8 = _fp8(np.ascontiguousarray(xpad.T), SX)
            blocks.append(
                np.ascontiguousarray(
                    x8.reshape(DC // 2, 2, P, C).transpose(2, 0, 1, 3)
                ).reshape(P, DC * C)
            )
        xc = np.concatenate(blocks, axis=1)

        wgu = np.stack(
            [
                np.stack([_prep_w1(_fp8(wg[e], SW)), _prep_w1(_fp8(wu[e], SW))])
                for e in (bins[c][sl] for sl in range(EPC))
            ]
        )
        wdc = np.stack([_prep_w2(_fp8(wd[bins[c][sl]], SW)) for sl in range(EPC)])

        half, g = c & 1, c >> 1
        sgh = sg[:, half * HSC : (half + 1) * HSC].astype(BF16)
        suh = su[:, half * HSC : (half + 1) * HSC].astype(BF16)
        sgc = np.stack(
            [
                _colmajor(np.ascontiguousarray(sgh[:, h * P : (h + 1) * P])).reshape(
                    P, DC * P
                )
                for h in range(HH)
            ]
        )
        suc = np.stack(
            [
                _colmajor(np.ascontiguousarray(suh[:, h * P : (h + 1) * P])).reshape(
                    P, DC * P
                )
                for h in range(HH)
            ]
        )
        sdh = sd[half * HSC : (half + 1) * HSC].astype(BF16)
        sdc = np.ascontiguousarray(
            sdh.reshape(HH, P, D).transpose(1, 0, 2).reshape(P, HH * D)
        )
        xgT = np.ascontiguousarray(xbf[g * SG : (g + 1) * SG].T)  # [D, SG]
        xtg = np.ascontiguousarray(
            xgT.reshape(4, 4, P, SG).transpose(0, 2, 1, 3).reshape(4, P, 4 * SG)
        )

        in_maps.append(
            {
                "xc": xc,
                "wgu": wgu,
                "wd": wdc,
                "xtg": xtg,
                "sgc": sgc,
                "suc": suc,
                "sdc": sdc,
            }
        )

    nc = _build_program(SC)
    import os

    trace = bool(os.environ.get("KERNEL_TRACE"))
    res = run_bass_kernel_spmd(nc, in_maps, list(range(NCORES)), trace=trace)
    global LAST_EXEC_NS
    LAST_EXEC_NS = res.exec_time_ns
    results = res.results

    # ---- combine (host: apply top-k weights + mixing coefficients) ----
    out = np.zeros((S, D), np.float32)
    for g in range(4):
        part = results[2 * g]["ysT"].astype(np.float32) + results[2 * g + 1][
            "ysT"
        ].astype(np.float32)
        ysh = part.reshape(P, DC, SG).transpose(1, 0, 2).reshape(D, SG).T  # [SG, D]
        out[g * SG : (g + 1) * SG] = ysh * coef[g * SG : (g + 1) * SG, 1:2]
    for c in range(NCORES):
        off = 0
        for sl in range(EPC):
            e = bins[c][sl]
            C = SC[sl]
            n = int(counts[e])
            blk = (
                results[c]["yT"][:, 16 * off : 16 * off + DC * C]
                .reshape(P, DC, C)
                .transpose(1, 0, 2)
                .reshape(D, C)
            )
            y = blk[:, :n].T.astype(np.float32) * wslot[e][:, None]
            out[tok_ids[e]] += y
            off += C
    return out.reshape(1, S, D).astype(np.float32)


# revision 15
# speedup vs baseline: 1.0526x; 1.0526x over previous
"""MoE block (KlearSparseMoeBlock) on 8 trn2 NeuronCores.

Strategy (expert-parallel, per sharding hint):
  - Host computes the (tiny, 0.3% of FLOPs) sigmoid router + top-4 and
    realizes the all-to-all token dispatch at the sharding step: each core
    gets pre-gathered, transposed activations for its 4 experts.
  - Experts are snake-assigned to cores by token count (rank r of 32 ->
    core via boustrophedon) so per-core slot totals balance to ~1030, and
    per-slot static capacities (296/272/256/248) hug the actual counts —
    ~5% padding vs 25% for a uniform 320 capacity.
  - Device (per core): expert SwiGLU in bf16 with fp32 PSUM accumulation.
    Both mm1 (up/gate) and mm2 (down) keep the model dim on PSUM
    partitions and tokens on the free axis, so matmul cost scales with
    actual token slots, not padded 128-tiles.
  - Shared expert is sharded 4 token groups x 2 hidden halves (cores
    2g, 2g+1 split HS): minimizes shared HBM traffic (10.5MB/core vs
    18.4MB for pure hidden-parallel).
  - No on-device combine scaling: expert outputs and shared partials
    return unscaled (transposed, bf16); the host applies top-k weights +
    mixing coefficients during the gather/unshard step. This removes all
    per-token DVE scaling work from the device critical path.
  - Input DMAs ride the SP HWDGE ring, output DMAs the Activation ring,
    so stores never head-block weight prefetches.
"""

import functools

import numpy as np
import ml_dtypes

BF16 = ml_dtypes.bfloat16
FP8 = ml_dtypes.float8_e3m4   # TRN FP8_EXP3: max normal +-15.5, 4 mantissa bits
FP8_LIM = 15.0

# fp8 scale factors (powers of two; unscaling folded into on-chip ops)
SW = 128.0    # expert weight scale (w ~ N(0, 0.02^2) -> +-14)
SX = 2.0      # token activation scale (x ~ N(0,1) -> +-10)
SA = 1.0      # swiglu activation scale (silu(g)*u, |.| < ~15)

# problem shapes (hardcoded per contract)
D = 2048      # model dim
H = 512       # expert hidden
E = 32        # experts
HS = 1024     # shared hidden
S = 2048      # tokens
NCORES = 8
EPC = E // NCORES          # experts per core = 4
P = 128
DC = D // P                # 16
HC = H // P                # 4
SG = S // 4                # shared token group = 512
HSC = HS // 2              # shared hidden half = 512
HH = HSC // P              # 4

# static slot capacities (snake by per-expert token count rank; tuned to
# the deterministic seed-0 routing: per-slot maxima 293/270/254/244)
DEFAULT_SC = (296, 272, 256, 256)


@functools.lru_cache(maxsize=2)
def _build_program(SC):
    import concourse.tile as tile
    from concourse import bacc, mybir

    f32 = mybir.dt.float32
    bf16 = mybir.dt.bfloat16
    silu = mybir.ActivationFunctionType.Silu
    CT = sum(SC)

    # Bacc (not raw Bass): its compile pipeline splits multi-sem waits into
    # event semaphores — TRN2 allows at most one wait per instruction.
    nc = bacc.Bacc(None)

    # ---- per-core inputs (all bf16, host-prepped partition-major) ----
    fp8 = mybir.dt.float8e3
    xc_d = nc.declare_dram_parameter("xc", [P, DC * CT], fp8, isOutput=False)
    wgu_d = nc.declare_dram_parameter(
        "wgu", [EPC, 2, P, HC * DC * P], fp8, isOutput=False
    )
    wd_d = nc.declare_dram_parameter(
        "wd", [EPC, P, DC * HC * P], fp8, isOutput=False
    )
    xtg_d = nc.declare_dram_parameter("xtg", [4, P, 4 * SG], bf16, isOutput=False)
    sg_d = nc.declare_dram_parameter("sgc", [HH, P, DC * P], bf16, isOutput=False)
    su_d = nc.declare_dram_parameter("suc", [HH, P, DC * P], bf16, isOutput=False)
    sd_d = nc.declare_dram_parameter("sdc", [P, HH * D], bf16, isOutput=False)

    # ---- per-core outputs (transposed, unscaled) ----
    yT_d = nc.declare_dram_parameter("yT", [P, DC * CT], bf16, isOutput=True)
    ysT_d = nc.declare_dram_parameter("ysT", [P, DC * SG], bf16, isOutput=True)

    SCMAX = max(SC)

    with tile.TileContext(nc) as tc:
        with (
            tc.tile_pool(name="shpool", bufs=1) as shpool,
            tc.tile_pool(name="wpool", bufs=4) as wpool,
            tc.tile_pool(name="xcpool", bufs=1) as xcpool,
            tc.tile_pool(name="apool", bufs=2) as apool,
            tc.tile_pool(name="tpool", bufs=2) as tpool,
            tc.tile_pool(name="ypool", bufs=4) as ypool,
            tc.tile_pool(name="mm1", bufs=2, space="PSUM") as mm1,
            tc.tile_pool(name="mm2", bufs=4, space="PSUM") as mm2,
        ):
            # ---------------- phase A: shared-expert mm1 ----------------
            # (runs first: tiny DMA footprint per PE-second, so expert
            # weights can stream in behind it)
            sght = [None] + [
                shpool.tile([P, DC * P], bf16, tag=f"sght{h}", name=f"sght{h}")
                for h in range(1, HH)
            ]
            suht = [shpool.tile([P, DC * P], bf16, tag=f"suht{h}", name=f"suht{h}") for h in range(HH)]
            xtg = [None] + [
                shpool.tile([P, 4, SG], bf16, tag=f"xtg{k}", name=f"xtg{k}")
                for k in range(1, 4)
            ]
            asT = shpool.tile([P, HH, SG], bf16, tag="asT")

            # fragment the first chunks so matmul 0 depends on 0.26MB
            sg0a = shpool.tile([P, 4 * P], bf16, tag="sg0a")
            sg0b = shpool.tile([P, 12 * P], bf16, tag="sg0b")
            xt0a = shpool.tile([P, 1, SG], bf16, tag="xt0a")
            xt0b = shpool.tile([P, 3, SG], bf16, tag="xt0b")
            nc.sync.dma_start(out=sg0a[:], in_=sg_d[0, :, : 4 * P])
            nc.sync.dma_start(out=xt0a[:], in_=xtg_d[0, :, :SG])
            nc.sync.dma_start(out=xt0b[:], in_=xtg_d[0, :, SG:])
            nc.sync.dma_start(out=sg0b[:], in_=sg_d[0, :, 4 * P :])
            for k in range(1, 4):
                nc.sync.dma_start(out=xtg[k][:], in_=xtg_d[k])
            nc.sync.dma_start(out=suht[0][:], in_=su_d[0])
            for h in range(1, HH):
                nc.sync.dma_start(out=sght[h][:], in_=sg_d[h])
                nc.sync.dma_start(out=suht[h][:], in_=su_d[h])

            for h in range(HH):
                pg = mm1.tile([P, 512], f32, tag="pg")
                pu = mm1.tile([P, 512], f32, tag="pu")
                for k in range(DC):
                    if h == 0:
                        lhs = sg0a[:, k * P : (k + 1) * P] if k < 4 else sg0b[
                            :, (k - 4) * P : (k - 3) * P
                        ]
                    else:
                        lhs = sght[h][:, k * P : (k + 1) * P]
                    if k == 0:
                        rhs = xt0a[:, 0, :]
                    elif k < 4:
                        rhs = xt0b[:, k - 1, :]
                    else:
                        rhs = xtg[k // 4][:, k % 4, :]
                    nc.tensor.matmul(
                        out=pg[:, :SG],
                        lhsT=lhs,
                        rhs=rhs,
                        start=(k == 0),
                        stop=(k == DC - 1),
                    )
                for k in range(DC):
                    if k == 0:
                        rhs = xt0a[:, 0, :]
                    elif k < 4:
                        rhs = xt0b[:, k - 1, :]
                    else:
                        rhs = xtg[k // 4][:, k % 4, :]
                    nc.tensor.matmul(
                        out=pu[:, :SG],
                        lhsT=suht[h][:, k * P : (k + 1) * P],
                        rhs=rhs,
                        start=(k == 0),
                        stop=(k == DC - 1),
                    )
                sg_t = tpool.tile([P, 512], bf16, tag="sgt")
                nc.scalar.activation(out=sg_t[:, :SG], in_=pg[:, :SG], func=silu)
                nc.vector.tensor_tensor(
                    out=asT[:, h, :],
                    in0=pu[:, :SG],
                    in1=sg_t[:, :SG],
                    op=mybir.AluOpType.mult,
                )

            # ---------------- expert + shared-mm2 interleave ----------------
            sd_sb = shpool.tile([P, HH, D], bf16, tag="sd")

            def shared_mm2(dd_range):
                for dd in dd_range:
                    ps = mm2.tile([P, 512], f32, tag="py")
                    for h in range(HH):
                        nc.tensor.matmul(
                            out=ps[:, :SG],
                            lhsT=sd_sb[:, h, dd * P : (dd + 1) * P],
                            rhs=asT[:, h, :],
                            start=(h == 0),
                            stop=(h == HH - 1),
                        )
                    ysh = ypool.tile([P, 512], bf16, tag="ysh")
                    if dd % 2 == 0:
                        nc.vector.tensor_copy(ysh[:, :SG], ps[:, :SG])
                    else:
                        nc.scalar.activation(
                            out=ysh[:, :SG], in_=ps[:, :SG],
                            func=mybir.ActivationFunctionType.Copy,
                        )
                    nc.sync.dma_start(
                        out=ysT_d[:, dd * SG : (dd + 1) * SG], in_=ysh[:, :SG]
                    )

            off = 0
            for e in range(EPC):
                C = SC[e]
                wg_sb = wpool.tile([P, HC, DC // 2, 2, P], fp8, tag="wgu")
                wu_sb = wpool.tile([P, HC, DC // 2, 2, P], fp8, tag="wgu")
                wd_sb = wpool.tile([P, DC, HC // 2, 2, P], fp8, tag="wd")
                xc_sb = xcpool.tile([P, DC // 2, 2, C], fp8, tag=f"xc{e}")
                nc.sync.dma_start(
                    out=wg_sb[:], in_=wgu_d[e, 0]
                )
                nc.sync.dma_start(
                    out=xc_sb[:], in_=xc_d[:, 16 * off : 16 * off + DC * C]
                )
                nc.sync.dma_start(
                    out=wu_sb[:], in_=wgu_d[e, 1]
                )
                nc.sync.dma_start(out=wd_sb[:], in_=wd_d[e])
                if e == 1:
                    nc.sync.dma_start(out=sd_sb[:], in_=sd_d[:])

                acT = apool.tile([P, HC // 2, 2, SCMAX], fp8, tag="acT")
                for hc in range(HC):
                    pg = mm1.tile([P, 512], f32, tag="pg")
                    pu = mm1.tile([P, 512], f32, tag="pu")
                    for k in range(DC):
                        nc.tensor.matmul(
                            out=pg[:, :C],
                            lhsT=wg_sb[:, hc, k // 2, k % 2, :],
                            rhs=xc_sb[:, k // 2, k % 2, :],
                            start=(k == 0),
                            stop=(k == DC - 1),
                        )
                    for k in range(DC):
                        nc.tensor.matmul(
                            out=pu[:, :C],
                            lhsT=wu_sb[:, hc, k // 2, k % 2, :],
                            rhs=xc_sb[:, k // 2, k % 2, :],
                            start=(k == 0),
                            stop=(k == DC - 1),
                        )
                    sg_t = tpool.tile([P, 512], bf16, tag="sgt")
                    # pg holds SW*SX*g; silu(scale*pg) = silu(g)
                    nc.scalar.activation(
                        out=sg_t[:, :C], in_=pg[:, :C], func=silu,
                        scale=1.0 / (SW * SX),
                    )
                    # acT_q = (pu * SA/(SW*SX)) * silu(g)  -> fp8, scale SA
                    nc.vector.scalar_tensor_tensor(
                        out=acT[:, hc // 2, hc % 2, :C],
                        in0=pu[:, :C],
                        scalar=SA / (SW * SX),
                        in1=sg_t[:, :C],
                        op0=mybir.AluOpType.mult,
                        op1=mybir.AluOpType.mult,
                    )

                for dd0 in range(0, DC, 2):
                    y_sb = ypool.tile([P, 2, SCMAX], bf16, tag="y")
                    for dd in (dd0, dd0 + 1):
                        py = mm2.tile([P, 512], f32, tag="py")
                        for j in range(HC):
                            nc.tensor.matmul(
                                out=py[:, :C],
                                lhsT=wd_sb[:, dd, j // 2, j % 2, :],
                                rhs=acT[:, j // 2, j % 2, :C],
                                start=(j == 0),
                                stop=(j == HC - 1),
                            )
                        # py holds SA*SW*y
                        if dd % 2 == 0:
                            nc.vector.tensor_scalar_mul(
                                y_sb[:, 0, :C], py[:, :C], 1.0 / (SA * SW)
                            )
                        else:
                            nc.scalar.activation(
                                out=y_sb[:, 1, :C], in_=py[:, :C],
                                func=mybir.ActivationFunctionType.Copy,
                                scale=1.0 / (SA * SW),
                            )
                    nc.sync.dma_start(
                        out=yT_d[
                            :, 16 * off + dd0 * C : 16 * off + (dd0 + 2) * C
                        ],
                        in_=y_sb[:, :, :C],
                    )
                off += C

                if e == 1:
                    shared_mm2(range(0, 8))
                elif e == 2:
                    shared_mm2(range(8, DC))

    if not nc.is_finalized():
        nc.finalize()  # Bacc: runs compile() (reg alloc, event-sem wait split)
    return nc


def _fp8(a, scale):
    return np.clip(a * scale, -FP8_LIM, FP8_LIM).astype(FP8)


def _prep_w1(w):
    # [D, H] -> [128, HC, DC/2, 2, 128]: [p, hc, j, pair, m] = w[(2j+pair)*128+p, hc*128+m]
    return np.ascontiguousarray(
        w.reshape(DC // 2, 2, P, HC, P)
        .transpose(2, 3, 0, 1, 4)
        .reshape(P, HC * DC * P)
    )


def _prep_w2(w):
    # [H, D] -> [128, DC, HC/2, 2, 128]: [p, dd, j, pair, m] = w[(2j+pair)*128+p, dd*128+m]
    return np.ascontiguousarray(
        w.reshape(HC // 2, 2, P, DC, P)
        .transpose(2, 3, 0, 1, 4)
        .reshape(P, DC * HC * P)
    )


def _colmajor(a):
    # [D, N] -> [128, DC, N]: [p, k, t] = a[k*128+p, t]
    n = a.shape[1]
    return np.ascontiguousarray(a.reshape(DC, P, n).transpose(1, 0, 2))


def kernel(x, gate_w, expert_bias, wg, wu, wd, sg, su, sd, coef_w, coef_b, top_k):
    from concourse.bass_utils import run_bass_kernel_spmd

    x2 = np.ascontiguousarray(np.asarray(x, dtype=np.float32).reshape(S, D))
    gate_w = np.asarray(gate_w, dtype=np.float32)
    expert_bias = np.asarray(expert_bias, dtype=np.float32)
    coef_w = np.asarray(coef_w, dtype=np.float32)
    coef_b = np.asarray(coef_b, dtype=np.float32)
    wg = np.asarray(wg, dtype=np.float32)
    wu = np.asarray(wu, dtype=np.float32)
    wd = np.asarray(wd, dtype=np.float32)
    sg = np.asarray(sg, dtype=np.float32)
    su = np.asarray(su, dtype=np.float32)
    sd = np.asarray(sd, dtype=np.float32)
    top_k = int(top_k)

    # ---- router (host; 0.3% of total FLOPs) ----
    logits = x2 @ gate_w.T
    routing = 1.0 / (1.0 + np.exp(-logits))
    biased = routing + expert_bias[None, :]
    inds = np.argpartition(-biased, top_k - 1, axis=-1)[:, :top_k]  # [S,K]
    scores = np.take_along_axis(routing, inds, axis=-1)
    wnorm = scores / (scores.sum(-1, keepdims=True) + 1e-20)

    cl = x2 @ coef_w.T + coef_b[None, :]
    cl -= cl.max(-1, keepdims=True)
    ce = np.exp(cl)
    coef = ce / ce.sum(-1, keepdims=True)  # [S,2]

    # ---- expert -> core assignment (snake by token-count rank) ----
    tok_ids = []
    wslot = []
    counts = np.zeros(E, np.int64)
    for e in range(E):
        rows, cols = np.nonzero(inds == e)
        counts[e] = len(rows)
        tok_ids.append(rows)
        wslot.append(wnorm[rows, cols] * coef[rows, 0])
    order = np.argsort(-counts, kind="stable")
    bins = [
        [int(order[b]), int(order[15 - b]), int(order[16 + b]), int(order[31 - b])]
        for b in range(NCORES)
    ]
    need = [max(counts[bins[b][sl]] for b in range(NCORES)) for sl in range(EPC)]
    if all(n <= c for n, c in zip(need, DEFAULT_SC)):
        SC = DEFAULT_SC
    else:
        SC = tuple(int(-(-n // 8) * 8) for n in need)  # round up to mult of 8
    CT = sum(SC)

    xbf = x2.astype(BF16)

    in_maps = []
    for c in range(NCORES):
        blocks = []
        for sl in range(EPC):
            e = bins[c][sl]
            C = SC[sl]
            n = int(counts[e])
            xpad = np.zeros((C, D), np.float32)
            xpad[:n] = x2[tok_ids[e]]
            # [D, C] -> [128, DC/2, 2, C]: [p, j, pair, t] = xT[(2j+pair)*128+p, t]
            x8 = _fp8(np.ascontiguousarray(xpad.T), SX)
            blocks.append(
                np.ascontiguousarray(
                    x8.reshape(DC // 2, 2, P, C).transpose(2, 0, 1, 3)
                ).reshape(P, DC * C)
            )
        xc = np.concatenate(blocks, axis=1)

        wgu = np.stack(
            [
                np.stack([_prep_w1(_fp8(wg[e], SW)), _prep_w1(_fp8(wu[e], SW))])
                for e in (bins[c][sl] for sl in range(EPC))
            ]
        )
        wdc = np.stack([_prep_w2(_fp8(wd[bins[c][sl]], SW)) for sl in range(EPC)])

        half, g = c & 1, c >> 1
        sgh = sg[:, half * HSC : (half + 1) * HSC].astype(BF16)
        suh = su[:, half * HSC : (half + 1) * HSC].astype(BF16)
        sgc = np.stack(
            [
                _colmajor(np.ascontiguousarray(sgh[:, h * P : (h + 1) * P])).reshape(
                    P, DC * P
                )
                for h in range(HH)
            ]
        )
        suc = np.stack(
            [
                _colmajor(np.ascontiguousarray(suh[:, h * P : (h + 1) * P])).reshape(
                    P, DC * P
                )
                for h in range(HH)
            ]
        )
        sdh = sd[half * HSC : (half + 1) * HSC].astype(BF16)
        sdc = np.ascontiguousarray(
            sdh.reshape(HH, P, D).transpose(1, 0, 2).reshape(P, HH * D)
        )
        xgT = np.ascontiguousarray(xbf[g * SG : (g + 1) * SG].T)  # [D, SG]
        xtg = np.ascontiguousarray(
            xgT.reshape(4, 4, P, SG).transpose(0, 2, 1, 3).reshape(4, P, 4 * SG)
        )

        in_maps.append(
            {
                "xc": xc,
                "wgu": wgu,
                "wd": wdc,
                "xtg": xtg,
                "sgc": sgc,
                "suc": suc,
                "sdc": sdc,
            }
        )

    nc = _build_program(SC)
    import os

    trace = bool(os.environ.get("KERNEL_TRACE"))
    res = run_bass_kernel_spmd(nc, in_maps, list(range(NCORES)), trace=trace)
    global LAST_EXEC_NS
    LAST_EXEC_NS = res.exec_time_ns
    results = res.results

    # ---- combine (host: apply top-k weights + mixing coefficients) ----
    out = np.zeros((S, D), np.float32)
    for g in range(4):
        part = results[2 * g]["ysT"].astype(np.float32) + results[2 * g + 1][
            "ysT"
        ].astype(np.float32)
        ysh = part.reshape(P, DC, SG).transpose(1, 0, 2).reshape(D, SG).T  # [SG, D]
        out[g * SG : (g + 1) * SG] = ysh * coef[g * SG : (g + 1) * SG, 1:2]
    for c in range(NCORES):
        off = 0
        for sl in range(EPC):
            e = bins[c][sl]
            C = SC[sl]
            n = int(counts[e])
            blk = (
                results[c]["yT"][:, 16 * off : 16 * off + DC * C]
                .reshape(P, DC, C)
                .transpose(1, 0, 2)
                .reshape(D, C)
            )
            y = blk[:, :n].T.astype(np.float32) * wslot[e][:, None]
            out[tok_ids[e]] += y
            off += C
    return out.reshape(1, S, D).astype(np.float32)


# revision 17
# speedup vs baseline: 1.0708x; 1.0173x over previous
"""MoE block (KlearSparseMoeBlock) on 8 trn2 NeuronCores.

Strategy (expert-parallel, per sharding hint):
  - Host computes the (tiny, 0.3% of FLOPs) sigmoid router + top-4 and
    realizes the all-to-all token dispatch at the sharding step: each core
    gets pre-gathered, transposed activations for its 4 experts.
  - Experts are snake-assigned to cores by token count (rank r of 32 ->
    core via boustrophedon) so per-core slot totals balance to ~1030, and
    per-slot static capacities (296/272/256/248) hug the actual counts —
    ~5% padding vs 25% for a uniform 320 capacity.
  - Device (per core): expert SwiGLU in bf16 with fp32 PSUM accumulation.
    Both mm1 (up/gate) and mm2 (down) keep the model dim on PSUM
    partitions and tokens on the free axis, so matmul cost scales with
    actual token slots, not padded 128-tiles.
  - Shared expert is sharded 4 token groups x 2 hidden halves (cores
    2g, 2g+1 split HS): minimizes shared HBM traffic (10.5MB/core vs
    18.4MB for pure hidden-parallel).
  - No on-device combine scaling: expert outputs and shared partials
    return unscaled (transposed, bf16); the host applies top-k weights +
    mixing coefficients during the gather/unshard step. This removes all
    per-token DVE scaling work from the device critical path.
  - Input DMAs ride the SP HWDGE ring, output DMAs the Activation ring,
    so stores never head-block weight prefetches.
"""

import functools

import numpy as np
import ml_dtypes

BF16 = ml_dtypes.bfloat16
FP8 = ml_dtypes.float8_e3m4   # TRN FP8_EXP3: max normal +-15.5, 4 mantissa bits
FP8_LIM = 15.0

# fp8 scale factors (powers of two; unscaling folded into on-chip ops)
SW = 128.0    # expert weight scale (w ~ N(0, 0.02^2) -> +-14)
SX = 2.0      # token activation scale (x ~ N(0,1) -> +-10)
SA = 1.0      # swiglu activation scale (silu(g)*u, |.| < ~15)

# problem shapes (hardcoded per contract)
D = 2048      # model dim
H = 512       # expert hidden
E = 32        # experts
HS = 1024     # shared hidden
S = 2048      # tokens
NCORES = 8
EPC = E // NCORES          # experts per core = 4
P = 128
DC = D // P                # 16
HC = H // P                # 4
SG = S // 4                # shared token group = 512
HSC = HS // 2              # shared hidden half = 512
HH = HSC // P              # 4

# static slot capacities (snake by per-expert token count rank; tuned to
# the deterministic seed-0 routing: per-slot maxima 293/270/254/244)
DEFAULT_SC = (296, 272, 256, 256)


@functools.lru_cache(maxsize=2)
def _build_program(SC):
    import concourse.tile as tile
    from concourse import bacc, mybir

    f32 = mybir.dt.float32
    bf16 = mybir.dt.bfloat16
    silu = mybir.ActivationFunctionType.Silu
    CT = sum(SC)

    # Bacc (not raw Bass): its compile pipeline splits multi-sem waits into
    # event semaphores — TRN2 allows at most one wait per instruction.
    nc = bacc.Bacc(None)

    # ---- per-core inputs (all bf16, host-prepped partition-major) ----
    fp8 = mybir.dt.float8e3
    xc_d = nc.declare_dram_parameter("xc", [P, DC * CT], fp8, isOutput=False)
    wgu_d = nc.declare_dram_parameter(
        "wgu", [EPC, 2, P, HC * DC * P], fp8, isOutput=False
    )
    wd_d = nc.declare_dram_parameter(
        "wd", [EPC, P, DC * HC * P], fp8, isOutput=False
    )
    xtg_d = nc.declare_dram_parameter("xtg", [4, P, 4 * SG], bf16, isOutput=False)
    sg_d = nc.declare_dram_parameter("sgc", [HH, P, DC * P], bf16, isOutput=False)
    su_d = nc.declare_dram_parameter("suc", [HH, P, DC * P], bf16, isOutput=False)
    sd_d = nc.declare_dram_parameter("sdc", [P, HH * D], bf16, isOutput=False)

    # ---- per-core outputs (transposed, unscaled) ----
    yT_d = nc.declare_dram_parameter("yT", [P, DC * CT], bf16, isOutput=True)
    ysT_d = nc.declare_dram_parameter("ysT", [P, DC * SG], bf16, isOutput=True)

    SCMAX = max(SC)

    with tile.TileContext(nc) as tc:
        with (
            tc.tile_pool(name="shpool", bufs=1) as shpool,
            tc.tile_pool(name="wpool", bufs=4) as wpool,
            tc.tile_pool(name="xcpool", bufs=1) as xcpool,
            tc.tile_pool(name="apool", bufs=2) as apool,
            tc.tile_pool(name="tpool", bufs=2) as tpool,
            tc.tile_pool(name="ypool", bufs=4) as ypool,
            tc.tile_pool(name="mm1", bufs=2, space="PSUM") as mm1,
            tc.tile_pool(name="mm2", bufs=4, space="PSUM") as mm2,
        ):
            # ---------------- phase A: shared-expert mm1 ----------------
            # (runs first: tiny DMA footprint per PE-second, so expert
            # weights can stream in behind it)
            sght = [None] + [
                shpool.tile([P, DC * P], bf16, tag=f"sght{h}", name=f"sght{h}")
                for h in range(1, HH)
            ]
            suht = [shpool.tile([P, DC * P], bf16, tag=f"suht{h}", name=f"suht{h}") for h in range(HH)]
            xtg = [None] + [
                shpool.tile([P, 4, SG], bf16, tag=f"xtg{k}", name=f"xtg{k}")
                for k in range(1, 4)
            ]
            asT = shpool.tile([P, HH, SG], bf16, tag="asT")

            # expert-0 inputs load FIRST: fp8 stream is light (2.7MB per
            # 16us of PE), letting the wire build credit for shared mm1
            C0 = SC[0]
            wg_a = wpool.tile([P, 1, DC // 2, 2, P], fp8, tag="wg0a")
            wg_b = wpool.tile([P, HC - 1, DC // 2, 2, P], fp8, tag="wg0b")
            xc_a = xcpool.tile([P, DC // 4, 2, C0], fp8, tag="xc0a")
            xc_b = xcpool.tile([P, DC // 4, 2, C0], fp8, tag="xc0b")
            wu0_sb = wpool.tile([P, HC, DC // 2, 2, P], fp8, tag="wgu")
            wd0_sb = wpool.tile([P, DC, HC // 2, 2, P], fp8, tag="wd")
            nc.sync.dma_start(out=wg_a[:], in_=wgu_d[0, 0, :, : DC * P])
            nc.sync.dma_start(out=xc_a[:], in_=xc_d[:, : DC * C0 // 2])
            nc.sync.dma_start(out=xc_b[:], in_=xc_d[:, DC * C0 // 2 : DC * C0])
            nc.sync.dma_start(out=wg_b[:], in_=wgu_d[0, 0, :, DC * P :])
            nc.sync.dma_start(out=wu0_sb[:], in_=wgu_d[0, 1])
            nc.sync.dma_start(out=wd0_sb[:], in_=wd_d[0])

            # fragment the first chunks so matmul 0 depends on 0.26MB
            sg0a = shpool.tile([P, 4 * P], bf16, tag="sg0a")
            sg0b = shpool.tile([P, 12 * P], bf16, tag="sg0b")
            xt0a = shpool.tile([P, 1, SG], bf16, tag="xt0a")
            xt0b = shpool.tile([P, 3, SG], bf16, tag="xt0b")
            nc.sync.dma_start(out=sg0a[:], in_=sg_d[0, :, : 4 * P])
            nc.sync.dma_start(out=xt0a[:], in_=xtg_d[0, :, :SG])
            nc.sync.dma_start(out=xt0b[:], in_=xtg_d[0, :, SG:])
            nc.sync.dma_start(out=sg0b[:], in_=sg_d[0, :, 4 * P :])
            for k in range(1, 4):
                nc.sync.dma_start(out=xtg[k][:], in_=xtg_d[k])
            nc.sync.dma_start(out=suht[0][:], in_=su_d[0])
            for h in range(1, HH):
                nc.sync.dma_start(out=sght[h][:], in_=sg_d[h])
                nc.sync.dma_start(out=suht[h][:], in_=su_d[h])

            def shared_mm1():
              for h in range(HH):
                pg = mm1.tile([P, 512], f32, tag="pg")
                pu = mm1.tile([P, 512], f32, tag="pu")
                for k in range(DC):
                    if h == 0:
                        lhs = sg0a[:, k * P : (k + 1) * P] if k < 4 else sg0b[
                            :, (k - 4) * P : (k - 3) * P
                        ]
                    else:
                        lhs = sght[h][:, k * P : (k + 1) * P]
                    if k == 0:
                        rhs = xt0a[:, 0, :]
                    elif k < 4:
                        rhs = xt0b[:, k - 1, :]
                    else:
                        rhs = xtg[k // 4][:, k % 4, :]
                    nc.tensor.matmul(
                        out=pg[:, :SG],
                        lhsT=lhs,
                        rhs=rhs,
                        start=(k == 0),
                        stop=(k == DC - 1),
                    )
                for k in range(DC):
                    if k == 0:
                        rhs = xt0a[:, 0, :]
                    elif k < 4:
                        rhs = xt0b[:, k - 1, :]
                    else:
                        rhs = xtg[k // 4][:, k % 4, :]
                    nc.tensor.matmul(
                        out=pu[:, :SG],
                        lhsT=suht[h][:, k * P : (k + 1) * P],
                        rhs=rhs,
                        start=(k == 0),
                        stop=(k == DC - 1),
                    )
                sg_t = tpool.tile([P, 512], bf16, tag="sgt")
                nc.scalar.activation(out=sg_t[:, :SG], in_=pg[:, :SG], func=silu)
                nc.vector.tensor_tensor(
                    out=asT[:, h, :],
                    in0=pu[:, :SG],
                    in1=sg_t[:, :SG],
                    op=mybir.AluOpType.mult,
                )

            # ---------------- expert + shared-mm2 interleave ----------------
            sd_sb = shpool.tile([P, HH, D], bf16, tag="sd")

            def shared_mm2(dd_range):
                for dd in dd_range:
                    ps = mm2.tile([P, 512], f32, tag="py")
                    for h in range(HH):
                        nc.tensor.matmul(
                            out=ps[:, :SG],
                            lhsT=sd_sb[:, h, dd * P : (dd + 1) * P],
                            rhs=asT[:, h, :],
                            start=(h == 0),
                            stop=(h == HH - 1),
                        )
                    ysh = ypool.tile([P, 512], bf16, tag="ysh")
                    if dd % 2 == 0:
                        nc.vector.tensor_copy(ysh[:, :SG], ps[:, :SG])
                    else:
                        nc.scalar.activation(
                            out=ysh[:, :SG], in_=ps[:, :SG],
                            func=mybir.ActivationFunctionType.Copy,
                        )
                    nc.sync.dma_start(
                        out=ysT_d[:, dd * SG : (dd + 1) * SG], in_=ysh[:, :SG]
                    )

            def expert(e, off):
                C = SC[e]
                if e == 0:
                    wu_sb, wd_sb = wu0_sb, wd0_sb

                    def wg_ap(hc, j):
                        t = wg_a if hc == 0 else wg_b
                        return t[:, hc if hc == 0 else hc - 1, j, :, :]

                    def xc_ap(j):
                        t = xc_a if j < DC // 4 else xc_b
                        return t[:, j if j < DC // 4 else j - DC // 4, :, :]
                else:
                    wg_sb = wpool.tile([P, HC, DC // 2, 2, P], fp8, tag="wgu")
                    xc_sb = xcpool.tile([P, DC // 2, 2, C], fp8, tag=f"xc{e}")
                    wu_sb = wpool.tile([P, HC, DC // 2, 2, P], fp8, tag="wgu")
                    wd_sb = wpool.tile([P, DC, HC // 2, 2, P], fp8, tag="wd")
                    nc.sync.dma_start(out=wg_sb[:], in_=wgu_d[e, 0])
                    nc.sync.dma_start(
                        out=xc_sb[:], in_=xc_d[:, 16 * off : 16 * off + DC * C]
                    )
                    nc.sync.dma_start(out=wu_sb[:], in_=wgu_d[e, 1])
                    nc.sync.dma_start(out=wd_sb[:], in_=wd_d[e])

                    def wg_ap(hc, j):
                        return wg_sb[:, hc, j, :, :]

                    def xc_ap(j):
                        return xc_sb[:, j, :, :]

                if e == 1:
                    nc.sync.dma_start(out=sd_sb[:], in_=sd_d[:])

                acT = apool.tile([P, HC // 2, 2, SCMAX], fp8, tag="acT")
                for hc in range(HC):
                    pg = mm1.tile([P, 512], f32, tag="pg")
                    pu = mm1.tile([P, 512], f32, tag="pu")
                    for k in range(DC):
                        nc.tensor.matmul(
                            out=pg[:, :C],
                            lhsT=wg_ap(hc, k // 2)[:, k % 2, :],
                            rhs=xc_ap(k // 2)[:, k % 2, :],
                            start=(k == 0),
                            stop=(k == DC - 1),
                        )
                    for k in range(DC):
                        nc.tensor.matmul(
                            out=pu[:, :C],
                            lhsT=wu_sb[:, hc, k // 2, k % 2, :],
                            rhs=xc_ap(k // 2)[:, k % 2, :],
                            start=(k == 0),
                            stop=(k == DC - 1),
                        )
                    sg_t = tpool.tile([P, 512], bf16, tag="sgt")
                    # pg holds SW*SX*g; silu(scale*pg) = silu(g)
                    nc.scalar.activation(
                        out=sg_t[:, :C], in_=pg[:, :C], func=silu,
                        scale=1.0 / (SW * SX),
                    )
                    # acT_q = (pu * SA/(SW*SX)) * silu(g)  -> fp8, scale SA
                    nc.vector.scalar_tensor_tensor(
                        out=acT[:, hc // 2, hc % 2, :C],
                        in0=pu[:, :C],
                        scalar=SA / (SW * SX),
                        in1=sg_t[:, :C],
                        op0=mybir.AluOpType.mult,
                        op1=mybir.AluOpType.mult,
                    )

                for dd0 in range(0, DC, 2):
                    y_sb = ypool.tile([P, 2, SCMAX], bf16, tag="y")
                    for dd in (dd0, dd0 + 1):
                        py = mm2.tile([P, 512], f32, tag="py")
                        for j in range(HC):
                            nc.tensor.matmul(
                                out=py[:, :C],
                                lhsT=wd_sb[:, dd, j // 2, j % 2, :],
                                rhs=acT[:, j // 2, j % 2, :C],
                                start=(j == 0),
                                stop=(j == HC - 1),
                            )
                        # py holds SA*SW*y
                        if dd % 2 == 0:
                            nc.vector.tensor_scalar_mul(
                                y_sb[:, 0, :C], py[:, :C], 1.0 / (SA * SW)
                            )
                        else:
                            nc.scalar.activation(
                                out=y_sb[:, 1, :C], in_=py[:, :C],
                                func=mybir.ActivationFunctionType.Copy,
                                scale=1.0 / (SA * SW),
                            )
                    nc.sync.dma_start(
                        out=yT_d[
                            :, 16 * off + dd0 * C : 16 * off + (dd0 + 2) * C
                        ],
                        in_=y_sb[:, :, :C],
                    )
            expert(0, 0)
            shared_mm1()
            expert(1, SC[0])
            shared_mm2(range(0, 8))
            expert(2, SC[0] + SC[1])
            shared_mm2(range(8, DC))
            expert(3, SC[0] + SC[1] + SC[2])

    if not nc.is_finalized():
        nc.finalize()  # Bacc: runs compile() (reg alloc, event-sem wait split)
    return nc


def _fp8(a, scale):
    return np.clip(a * scale, -FP8_LIM, FP8_LIM).astype(FP8)


def _prep_w1(w):
    # [D, H] -> [128, HC, DC/2, 2, 128]: [p, hc, j, pair, m] = w[(2j+pair)*128+p, hc*128+m]
    return np.ascontiguousarray(
        w.reshape(DC // 2, 2, P, HC, P)
        .transpose(2, 3, 0, 1, 4)
        .reshape(P, HC * DC * P)
    )


def _prep_w2(w):
    # [H, D] -> [128, DC, HC/2, 2, 128]: [p, dd, j, pair, m] = w[(2j+pair)*128+p, dd*128+m]
    return np.ascontiguousarray(
        w.reshape(HC // 2, 2, P, DC, P)
        .transpose(2, 3, 0, 1, 4)
        .reshape(P, DC * HC * P)
    )


def _colmajor(a):
    # [D, N] -> [128, DC, N]: [p, k, t] = a[k*128+p, t]
    n = a.shape[1]
    return np.ascontiguousarray(a.reshape(DC, P, n).transpose(1, 0, 2))


def kernel(x, gate_w, expert_bias, wg, wu, wd, sg, su, sd, coef_w, coef_b, top_k):
    from concourse.bass_utils import run_bass_kernel_spmd

    x2 = np.ascontiguousarray(np.asarray(x, dtype=np.float32).reshape(S, D))
    gate_w = np.asarray(gate_w, dtype=np.float32)
    expert_bias = np.asarray(expert_bias, dtype=np.float32)
    coef_w = np.asarray(coef_w, dtype=np.float32)
    coef_b = np.asarray(coef_b, dtype=np.float32)
    wg = np.asarray(wg, dtype=np.float32)
    wu = np.asarray(wu, dtype=np.float32)
    wd = np.asarray(wd, dtype=np.float32)
    sg = np.asarray(sg, dtype=np.float32)
    su = np.asarray(su, dtype=np.float32)
    sd = np.asarray(sd, dtype=np.float32)
    top_k = int(top_k)

    # ---- router (host; 0.3% of total FLOPs) ----
    logits = x2 @ gate_w.T
    routing = 1.0 / (1.0 + np.exp(-logits))
    biased = routing + expert_bias[None, :]
    inds = np.argpartition(-biased, top_k - 1, axis=-1)[:, :top_k]  # [S,K]
    scores = np.take_along_axis(routing, inds, axis=-1)
    wnorm = scores / (scores.sum(-1, keepdims=True) + 1e-20)

    cl = x2 @ coef_w.T + coef_b[None, :]
    cl -= cl.max(-1, keepdims=True)
    ce = np.exp(cl)
    coef = ce / ce.sum(-1, keepdims=True)  # [S,2]

    # ---- expert -> core assignment (snake by token-count rank) ----
    tok_ids = []
    wslot = []
    counts = np.zeros(E, np.int64)
    for e in range(E):
        rows, cols = np.nonzero(inds == e)
        counts[e] = len(rows)
        tok_ids.append(rows)
        wslot.append(wnorm[rows, cols] * coef[rows, 0])
    order = np.argsort(-counts, kind="stable")
    bins = [
        [int(order[b]), int(order[15 - b]), int(order[16 + b]), int(order[31 - b])]
        for b in range(NCORES)
    ]
    need = [max(counts[bins[b][sl]] for b in range(NCORES)) for sl in range(EPC)]
    if all(n <= c for n, c in zip(need, DEFAULT_SC)):
        SC = DEFAULT_SC
    else:
        SC = tuple(int(-(-n // 8) * 8) for n in need)  # round up to mult of 8
    CT = sum(SC)

    xbf = x2.astype(BF16)

    in_maps = []
    for c in range(NCORES):
        blocks = []
        for sl in range(EPC):
            e = bins[c][sl]
            C = SC[sl]
            n = int(counts[e])
            xpad = np.zeros((C, D), np.float32)
            xpad[:n] = x2[tok_ids[e]]
            # [D, C] -> [128, DC/2, 2, C]: [p, j, pair, t] = xT[(2j+pair)*128+p, t]
            x8 = _fp8(np.ascontiguousarray(xpad.T), SX)
            blocks.append(
                np.ascontiguousarray(
                    x8.reshape(DC // 2, 2, P, C).transpose(2, 0, 1, 3)
                ).reshape(P, DC * C)
            )
        xc = np.concatenate(blocks, axis=1)

        wgu = np.stack(
            [
                np.stack([_prep_w1(_fp8(wg[e], SW)), _prep_w1(_fp8(wu[e], SW))])
                for e in (bins[c][sl] for sl in range(EPC))
            ]
        )
        wdc = np.stack([_prep_w2(_fp8(wd[bins[c][sl]], SW)) for sl in range(EPC)])

        half, g = c & 1, c >> 1
        sgh = sg[:, half * HSC : (half + 1) * HSC].astype(BF16)
        suh = su[:, half * HSC : (half + 1) * HSC].astype(BF16)
        sgc = np.stack(
            [
                _colmajor(np.ascontiguousarray(sgh[:, h * P : (h + 1) * P])).reshape(
                    P, DC * P
                )
                for h in range(HH)
            ]
        )
        suc = np.stack(
            [
                _colmajor(np.ascontiguousarray(suh[:, h * P : (h + 1) * P])).reshape(
                    P, DC * P
                )
                for h in range(HH)
            ]
        )
        sdh = sd[half * HSC : (half + 1) * HSC].astype(BF16)
        sdc = np.ascontiguousarray(
            sdh.reshape(HH, P, D).transpose(1, 0, 2).reshape(P, HH * D)
        )
        xgT = np.ascontiguousarray(xbf[g * SG : (g + 1) * SG].T)  # [D, SG]
        xtg = np.ascontiguousarray(
            xgT.reshape(4, 4, P, SG).transpose(0, 2, 1, 3).reshape(4, P, 4 * SG)
        )

        in_maps.append(
            {
                "xc": xc,
                "wgu": wgu,
                "wd": wdc,
                "xtg": xtg,
                "sgc": sgc,
                "suc": suc,
                "sdc": sdc,
            }
        )

    nc = _build_program(SC)
    import os

    trace = bool(os.environ.get("KERNEL_TRACE"))
    res = run_bass_kernel_spmd(nc, in_maps, list(range(NCORES)), trace=trace)
    global LAST_EXEC_NS
    LAST_EXEC_NS = res.exec_time_ns
    results = res.results

    # ---- combine (host: apply top-k weights + mixing coefficients) ----
    out = np.zeros((S, D), np.float32)
    for g in range(4):
        part = results[2 * g]["ysT"].astype(np.float32) + results[2 * g + 1][
            "ysT"
        ].astype(np.float32)
        ysh = part.reshape(P, DC, SG).transpose(1, 0, 2).reshape(D, SG).T  # [SG, D]
        out[g * SG : (g + 1) * SG] = ysh * coef[g * SG : (g + 1) * SG, 1:2]
    for c in range(NCORES):
        off = 0
        for sl in range(EPC):
            e = bins[c][sl]
            C = SC[sl]
            n = int(counts[e])
            blk = (
                results[c]["yT"][:, 16 * off : 16 * off + DC * C]
                .reshape(P, DC, C)
                .transpose(1, 0, 2)
                .reshape(D, C)
            )
            y = blk[:, :n].T.astype(np.float32) * wslot[e][:, None]
            out[tok_ids[e]] += y
            off += C
    return out.reshape(1, S, D).astype(np.float32)
